# revision 15
# baseline (speedup 1.0000x reference)
"""DynamicConvolution TRN2 Bass kernel.

Problem (per reference):
  x: (32, 128, 64, 64) f32
  attention: pooled = mean(x, HW) -> MLP (relu) -> prompt dot -> softmax over K=8
  agg_w[b] = sum_k alpha[b,k] * kernels_weights[k]  (K=8 banks of (128,128,3,3))
  out[b] = conv2d(x[b], agg_w[b], pad=1) + agg_b[b]   -> (32, 128, 64, 64)

Strategy:
  - Data-parallel over batch: 8 cores x 4 samples.
  - Conv as 9 shifted matmuls (tap-wise) accumulating in PSUM, fp32r (TF32)
    matmuls at 1 col/cycle.  x is zero-padded to 66x66 on host, pre-rounded
    to TF32 (bit-exact with HW fp32r rounding), so every matmul is a full
    (128 x 512) tile: out rows = 8 image rows.
  - Attention MLP batched over the 4 local samples with plain fp32 matmuls
    (tiny).  Softmax on DVE/ACT.  alphas bounced through DRAM to get
    partition-broadcast + k-on-partition layouts.
  - Kernel aggregation: 8 scalar_tensor_tensor ops on DVE per sample
    (fp32), final round to fp32r for the PE.
"""
import sys

sys.path.insert(0, "/opt/trn_rl_repo")

import numpy as np

import concourse.bacc as bacc
import concourse.mybir as mybir
import concourse.tile as tile
from concourse.bass_utils import run_bass_kernel_spmd

# problem dims
B, C, H, W = 32, 128, 64, 64
K, KS = 8, 3
HID = 512
NCORES = 8
BL = B // NCORES          # local batch = 4
HP, WP = H + 2, W + 2     # 66x66 padded
NPIX = HP * WP            # 4356
RCHUNK = 8                # output rows per PSUM chunk
NCHUNK = H // RCHUNK      # 8
F32 = mybir.dt.float32
F32R = mybir.dt.float32r


def round_tf32(a: np.ndarray) -> np.ndarray:
    """Round-to-nearest-even to TF32 (10-bit mantissa) — matches HW fp32r."""
    a = np.ascontiguousarray(a, dtype=np.float32)
    u = a.view(np.uint32).astype(np.uint64)
    r = (u + 0xFFF + ((u >> 13) & 1)) & ~np.uint64(0x1FFF)
    return r.astype(np.uint32).view(np.float32)


def build(timing_chain: bool = False, probe_skip=()):
    """probe_skip: analysis-only knobs ('mlp', 'agg', 'reduce') that stub out
    pipeline stages so TimelineSim can attribute predicted time."""
    nc = bacc.Bacc("TRN2", target_bir_lowering=False, debug=False)

    if timing_chain:
        # unused input whose only purpose is to let a timing harness chain
        # iteration i's output into iteration i+1 (forces serial execution)
        nc.dram_tensor("chain", [BL, C, H * W], F32, kind="ExternalInput")
    xp = nc.dram_tensor("xp", [BL, C, NPIX], F32R, kind="ExternalInput")
    w1t = nc.dram_tensor("w1t", [C, HID], F32, kind="ExternalInput")
    b1c = nc.dram_tensor("b1c", [C, 4], F32, kind="ExternalInput")
    w2t = nc.dram_tensor("w2t", [C, 4, HID], F32, kind="ExternalInput")
    b2c = nc.dram_tensor("b2c", [C, 4], F32, kind="ExternalInput")
    pt = nc.dram_tensor("pt", [C, 4, K], F32, kind="ExternalInput")
    kb = nc.dram_tensor("kb", [K, C], F32, kind="ExternalInput")
    kw = nc.dram_tensor("kw", [C, K, KS * KS, C], F32, kind="ExternalInput")
    out = nc.dram_tensor("out", [BL, C, H * W], F32, kind="ExternalOutput")
    alpha_dram = nc.dram_tensor("alpha_scratch", [BL, K], F32)

    with tile.TileContext(nc) as tc:
        with (
            tc.tile_pool(name="singles", bufs=1) as singles,
            tc.tile_pool(name="xpool", bufs=BL) as xpool,
            tc.tile_pool(name="opool", bufs=2) as opool,
            tc.tile_pool(name="aggpool", bufs=2) as aggpool,
            tc.tile_pool(name="scr", bufs=1) as scr,
            tc.tile_pool(name="mlpp", bufs=1, space="PSUM") as mlpp,
            tc.tile_pool(
                name="convp", bufs=8 if "psum8" in probe_skip else 4, space="PSUM"
            ) as convp,
        ):
            # ---- load x first (padded, fp32r): pooled reduces gate the MLP ----
            x_sb = []
            for s in range(BL):
                xt = xpool.tile([C, HP, WP], F32R, tag="x")
                nc.sync.dma_start(
                    out=xt, in_=xp.ap()[s].rearrange("p (a b) -> p a b", a=HP)
                )
                x_sb.append(xt)

            # ---- load weights ----
            w1t_sb = singles.tile([C, HID], F32)
            nc.sync.dma_start(out=w1t_sb, in_=w1t.ap())
            b1_sb = singles.tile([C, 4], F32)
            nc.sync.dma_start(out=b1_sb, in_=b1c.ap())
            w2t_sb = singles.tile([C, 4, HID], F32)
            nc.sync.dma_start(out=w2t_sb, in_=w2t.ap())
            b2_sb = singles.tile([C, 4], F32)
            nc.sync.dma_start(out=b2_sb, in_=b2c.ap())
            pt_sb = singles.tile([C, 4, K], F32)
            nc.sync.dma_start(out=pt_sb, in_=pt.ap())
            kb_sb = singles.tile([K, C], F32)
            nc.sync.dma_start(out=kb_sb, in_=kb.ap())
            # kernel bank split per-k so aggregation isn't gated on one 4.7MB DMA
            kw_sb = singles.tile([C, K, KS * KS, C], F32)
            for k in range(K):
                nc.sync.dma_start(out=kw_sb[:, k], in_=kw.ap()[:, k])

            # ---- pooled sums (mean folded into relu scale later) ----
            # split across DVE and ACT so the 4 reduces serialize half as long
            pooled = singles.tile([C, BL], F32)
            junk = singles.tile([C, NPIX], F32)
            if "reduce" in probe_skip:
                nc.vector.memset(pooled, 1.0)
            else:
                for s in range(BL):
                    if s % 2 == 0:
                        nc.vector.tensor_reduce(
                            pooled[:, s : s + 1],
                            x_sb[s].bitcast(F32),
                            axis=mybir.AxisListType.XY,
                            op=mybir.AluOpType.add,
                        )
                    else:
                        nc.scalar.activation(
                            junk,
                            x_sb[s].bitcast(F32).rearrange("p a b -> p (a b)"),
                            mybir.ActivationFunctionType.Copy,
                            accum_out=pooled[:, s : s + 1],
                        )

            # ---- MLP: h = relu(W1 @ mean + b1) ----
            skip_mlp = "mlp" in probe_skip
            h_sb = singles.tile([C, 4, BL], F32)
            if skip_mlp:
                alpha_bc = singles.tile([C, BL, K], F32)
                nc.vector.memset(alpha_bc, 0.125)
                alpha_k8 = singles.tile([K, BL], F32)
                nc.vector.memset(alpha_k8, 0.125)
                aggb_sb = singles.tile([C, BL], F32)
                nc.vector.memset(aggb_sb, 0.0)
            ps_h = None if skip_mlp else mlpp.tile([C, 4, BL], F32, tag="ps_h")
            for c in range(4) if not skip_mlp else []:
                nc.tensor.matmul(
                    ps_h[:, c, :], w1t_sb[:, 128 * c : 128 * (c + 1)], pooled,
                    start=True, stop=True,
                )
                nc.scalar.activation(
                    h_sb[:, c, :], ps_h[:, c, :],
                    mybir.ActivationFunctionType.Relu,
                    bias=b1_sb[:, c : c + 1], scale=1.0 / (H * W),
                )

            # ---- s = W2 @ h + b2 ----
            s_sb = singles.tile([C, 4, BL], F32)
            ps_s = None if skip_mlp else mlpp.tile([C, 4, BL], F32, tag="ps_s")
            for c2 in range(4) if not skip_mlp else []:
                for c in range(4):
                    nc.tensor.matmul(
                        ps_s[:, c2, :],
                        w2t_sb[:, c, 128 * c2 : 128 * (c2 + 1)],
                        h_sb[:, c, :],
                        start=(c == 0), stop=(c == 3),
                    )
                nc.scalar.activation(
                    s_sb[:, c2, :], ps_s[:, c2, :],
                    mybir.ActivationFunctionType.Identity,
                    bias=b2_sb[:, c2 : c2 + 1],
                )

            # ---- scores = s . prompt  -> (BL, K) ----
            ps_sc = None if skip_mlp else mlpp.tile([BL, K], F32, tag="ps_sc")
            for c2 in range(4) if not skip_mlp else []:
                nc.tensor.matmul(
                    ps_sc, s_sb[:, c2, :], pt_sb[:, c2, :],
                    start=(c2 == 0), stop=(c2 == 3),
                )

            if not skip_mlp:
                # ---- softmax over K (free dim), rows = samples ----
                negmx = scr.tile([BL, 1], F32)
                nc.vector.tensor_reduce(
                    negmx, ps_sc, axis=mybir.AxisListType.X,
                    op=mybir.AluOpType.max, negate=True,
                )
                ex = scr.tile([BL, K], F32)
                nc.scalar.activation(
                    ex, ps_sc, mybir.ActivationFunctionType.Exp, bias=negmx,
                )
                sm = scr.tile([BL, 1], F32)
                nc.vector.tensor_reduce(
                    sm, ex, axis=mybir.AxisListType.X, op=mybir.AluOpType.add
                )
                rsm = scr.tile([BL, 1], F32)
                nc.vector.reciprocal(rsm, sm)
                alphas = scr.tile([BL, K], F32)
                nc.vector.tensor_scalar_mul(alphas, ex, rsm)

                # ---- broadcast alphas via DRAM bounce ----
                nc.sync.dma_start(out=alpha_dram.ap(), in_=alphas)
                alpha_bc = singles.tile([C, BL, K], F32)
                nc.sync.dma_start(
                    out=alpha_bc,
                    in_=alpha_dram.ap().rearrange("b k -> (b k)").unsqueeze(0)
                    .to_broadcast((C, BL * K))
                    .rearrange("p (b k) -> p b k", b=BL),
                )
                alpha_k8 = singles.tile([K, BL], F32)
                nc.sync.dma_start(
                    out=alpha_k8, in_=alpha_dram.ap().rearrange("b k -> k b")
                )

                # ---- agg_b = alphas @ kernels_bias  -> (C_out, BL) ----
                ps_ab = mlpp.tile([C, BL], F32, tag="ps_ab")
                nc.tensor.matmul(ps_ab, kb_sb, alpha_k8, start=True, stop=True)
                aggb_sb = singles.tile([C, BL], F32)
                nc.scalar.copy(aggb_sb, ps_ab)

            # ---- per sample: aggregate kernel bank, conv, bias, store ----
            taps = [(ti, tj) for ti in range(KS) for tj in range(KS)]
            for s in range(BL):
                if "agg" in probe_skip:
                    aggw = aggpool.tile([C, KS * KS, C], F32R, tag="aggw")
                    nc.vector.tensor_copy(aggw, kw_sb[:, 0])
                    o_sb = opool.tile([C, H, W], F32, tag="out")
                    for chunk in range(NCHUNK):
                        h0 = chunk * RCHUNK
                        ps_c = convp.tile([C, RCHUNK, W], F32, tag="ps_c")
                        for t, (ti, tj) in enumerate(taps):
                            nc.tensor.matmul(
                                ps_c, aggw[:, t, :],
                                x_sb[s][:, h0 + ti : h0 + ti + RCHUNK, tj : tj + W],
                                start=(t == 0), stop=(t == KS * KS - 1),
                            )
                        if "evict_dve" in probe_skip:
                            nc.vector.tensor_scalar_add(
                                o_sb[:, h0 : h0 + RCHUNK, :], ps_c,
                                aggb_sb[:, s : s + 1],
                            )
                        else:
                            nc.scalar.activation(
                                o_sb[:, h0 : h0 + RCHUNK, :], ps_c,
                                mybir.ActivationFunctionType.Identity,
                                bias=aggb_sb[:, s : s + 1],
                            )
                    nc.sync.dma_start(
                        out=out.ap()[s], in_=o_sb.rearrange("p a b -> p (a b)")
                    )
                    continue
                # weighted sum of 8 kernel banks on DVE
                sA = aggpool.tile([C, KS * KS, C], F32, tag="aggA")
                sB = aggpool.tile([C, KS * KS, C], F32, tag="aggB")
                pp = [sA, sB]
                nc.vector.tensor_scalar_mul(
                    sA, kw_sb[:, 0], alpha_bc[:, s, 0:1]
                )
                for k in range(1, K - 1):
                    nc.vector.scalar_tensor_tensor(
                        pp[k % 2], kw_sb[:, k], alpha_bc[:, s, k : k + 1],
                        pp[(k + 1) % 2],
                        op0=mybir.AluOpType.mult, op1=mybir.AluOpType.add,
                    )
                aggw = aggpool.tile([C, KS * KS, C], F32R, tag="aggw")
                nc.vector.scalar_tensor_tensor(
                    aggw, kw_sb[:, K - 1], alpha_bc[:, s, K - 1 : K],
                    pp[(K - 2) % 2],
                    op0=mybir.AluOpType.mult, op1=mybir.AluOpType.add,
                )

                o_sb = opool.tile([C, H, W], F32, tag="out")
                for chunk in range(NCHUNK):
                    h0 = chunk * RCHUNK
                    ps_c = convp.tile([C, RCHUNK, W], F32, tag="ps_c")
                    for t, (ti, tj) in enumerate(taps):
                        nc.tensor.matmul(
                            ps_c,
                            aggw[:, t, :],
                            x_sb[s][:, h0 + ti : h0 + ti + RCHUNK, tj : tj + W],
                            start=(t == 0), stop=(t == KS * KS - 1),
                        )
                    nc.scalar.activation(
                        o_sb[:, h0 : h0 + RCHUNK, :], ps_c,
                        mybir.ActivationFunctionType.Identity,
                        bias=aggb_sb[:, s : s + 1],
                    )
                nc.sync.dma_start(
                    out=out.ap()[s], in_=o_sb.rearrange("p a b -> p (a b)")
                )

    nc.compile()
    return nc


_NC = None


def _get_nc():
    global _NC
    if _NC is None:
        _NC = build()
    return _NC


def prep_inputs(x, prompt_param, w1, b1, w2, b2, kernels_weights, kernels_bias):
    """Host-side layout transforms -> per-core in_maps."""
    x = np.asarray(x, np.float32)
    prompt = np.asarray(prompt_param, np.float32)[0]          # (K, HID)
    w1 = np.asarray(w1, np.float32)
    b1 = np.asarray(b1, np.float32)
    w2 = np.asarray(w2, np.float32)
    b2 = np.asarray(b2, np.float32)
    kwt = np.asarray(kernels_weights, np.float32)             # (K, C, C, 3, 3)
    kbt = np.asarray(kernels_bias, np.float32)                # (K, C)

    w1t = np.ascontiguousarray(w1.T)                          # (C, HID)
    b1c = np.ascontiguousarray(b1.reshape(4, C).T)            # (C, 4)
    w2t = np.ascontiguousarray(w2.T.reshape(4, C, HID).transpose(1, 0, 2))
    b2c = np.ascontiguousarray(b2.reshape(4, C).T)
    pt = np.ascontiguousarray(prompt.T.reshape(4, C, K).transpose(1, 0, 2))
    kw = np.ascontiguousarray(kwt.transpose(2, 0, 3, 4, 1).reshape(C, K, KS * KS, C))
    kb = np.ascontiguousarray(kbt)

    in_maps = []
    for c in range(NCORES):
        xs = x[c * BL : (c + 1) * BL]                          # (4, C, H, W)
        xpad = np.zeros((BL, C, HP, WP), np.float32)
        xpad[:, :, 1 : H + 1, 1 : W + 1] = xs
        xpad = round_tf32(xpad).reshape(BL, C, NPIX)
        in_maps.append(
            {
                "xp": xpad, "w1t": w1t, "b1c": b1c, "w2t": w2t, "b2c": b2c,
                "pt": pt, "kb": kb, "kw": kw,
            }
        )
    return in_maps


def kernel(**inputs) -> np.ndarray:
    nc = _get_nc()
    in_maps = prep_inputs(**inputs)
    res = run_bass_kernel_spmd(nc, in_maps, core_ids=list(range(NCORES)))
    outs = [res.results[c]["out"].reshape(BL, C, H, W) for c in range(NCORES)]
    return np.concatenate(outs, axis=0)


if __name__ == "__main__":
    import reference

    inputs = {k: np.asarray(v) for k, v in reference.setup_inputs().items()}
    expected = np.asarray(reference.reference(**inputs))
    actual = kernel(**inputs)
    scale = np.abs(expected).max()
    err = np.abs(actual - expected).max()
    print(f"absmax={err:.3e} scale={scale:.3f} rel={err / scale:.3e}")


# revision 17
# speedup vs baseline: 1.0044x; 1.0044x over previous
"""DynamicConvolution TRN2 Bass kernel.

Problem (per reference):
  x: (32, 128, 64, 64) f32
  attention: pooled = mean(x, HW) -> MLP (relu) -> prompt dot -> softmax over K=8
  agg_w[b] = sum_k alpha[b,k] * kernels_weights[k]  (K=8 banks of (128,128,3,3))
  out[b] = conv2d(x[b], agg_w[b], pad=1) + agg_b[b]   -> (32, 128, 64, 64)

Strategy:
  - Data-parallel over batch: 8 cores x 4 samples.
  - Conv as 9 shifted matmuls (tap-wise) accumulating in PSUM, fp32r (TF32)
    matmuls at 1 col/cycle.  x is zero-padded to 66x66 on host, pre-rounded
    to TF32 (bit-exact with HW fp32r rounding), so every matmul is a full
    (128 x 512) tile: out rows = 8 image rows.
  - Attention MLP batched over the 4 local samples with plain fp32 matmuls
    (tiny).  Softmax on DVE/ACT.  alphas bounced through DRAM to get
    partition-broadcast + k-on-partition layouts.
  - Kernel aggregation: 8 scalar_tensor_tensor ops on DVE per sample
    (fp32), final round to fp32r for the PE.
"""
import sys

sys.path.insert(0, "/opt/trn_rl_repo")

import numpy as np

import concourse.bacc as bacc
import concourse.mybir as mybir
import concourse.tile as tile
from concourse.bass_utils import run_bass_kernel_spmd

# problem dims
B, C, H, W = 32, 128, 64, 64
K, KS = 8, 3
HID = 512
NCORES = 8
BL = B // NCORES          # local batch = 4
HP, WP = H + 2, W + 2     # 66x66 padded
NPIX = HP * WP            # 4356
RCHUNK = 8                # output rows per PSUM chunk
NCHUNK = H // RCHUNK      # 8
F32 = mybir.dt.float32
F32R = mybir.dt.float32r


def round_tf32(a: np.ndarray) -> np.ndarray:
    """Round-to-nearest-even to TF32 (10-bit mantissa) — matches HW fp32r."""
    a = np.ascontiguousarray(a, dtype=np.float32)
    u = a.view(np.uint32).astype(np.uint64)
    r = (u + 0xFFF + ((u >> 13) & 1)) & ~np.uint64(0x1FFF)
    return r.astype(np.uint32).view(np.float32)


def build(timing_chain: bool = False, probe_skip=()):
    """probe_skip: analysis-only knobs ('mlp', 'agg', 'reduce') that stub out
    pipeline stages so TimelineSim can attribute predicted time."""
    nc = bacc.Bacc("TRN2", target_bir_lowering=False, debug=False)

    if timing_chain:
        # unused input whose only purpose is to let a timing harness chain
        # iteration i's output into iteration i+1 (forces serial execution)
        nc.dram_tensor("chain", [BL, C, H * W], F32, kind="ExternalInput")
    xp = nc.dram_tensor("xp", [BL, C, NPIX], F32R, kind="ExternalInput")
    w1t = nc.dram_tensor("w1t", [C, HID], F32, kind="ExternalInput")
    b1c = nc.dram_tensor("b1c", [C, 4], F32, kind="ExternalInput")
    w2t = nc.dram_tensor("w2t", [C, 4, HID], F32, kind="ExternalInput")
    b2c = nc.dram_tensor("b2c", [C, 4], F32, kind="ExternalInput")
    pt = nc.dram_tensor("pt", [C, 4, K], F32, kind="ExternalInput")
    kb = nc.dram_tensor("kb", [K, C], F32, kind="ExternalInput")
    kw = nc.dram_tensor("kw", [C, K, KS * KS, C], F32, kind="ExternalInput")
    out = nc.dram_tensor("out", [BL, C, H * W], F32, kind="ExternalOutput")
    alpha_dram = nc.dram_tensor("alpha_scratch", [BL, K], F32)

    with tile.TileContext(nc) as tc:
        with (
            tc.tile_pool(name="singles", bufs=1) as singles,
            tc.tile_pool(name="xpool", bufs=BL) as xpool,
            tc.tile_pool(name="opool", bufs=2) as opool,
            tc.tile_pool(name="aggpool", bufs=2) as aggpool,
            tc.tile_pool(name="scr", bufs=1) as scr,
            tc.tile_pool(name="mlpp", bufs=2, space="PSUM") as mlpp,
            tc.tile_pool(
                name="convp", bufs=8 if "psum8" in probe_skip else 4, space="PSUM"
            ) as convp,
        ):
            # ---- load x first (padded, fp32r): pooled reduces gate the MLP ----
            x_sb = []
            for s in range(BL):
                xt = xpool.tile([C, HP, WP], F32R, tag="x")
                nc.sync.dma_start(
                    out=xt, in_=xp.ap()[s].rearrange("p (a b) -> p a b", a=HP)
                )
                x_sb.append(xt)

            # ---- load weights ----
            w1t_sb = singles.tile([C, HID], F32)
            nc.sync.dma_start(out=w1t_sb, in_=w1t.ap())
            b1_sb = singles.tile([C, 4], F32)
            nc.sync.dma_start(out=b1_sb, in_=b1c.ap())
            w2t_sb = singles.tile([C, 4, HID], F32)
            nc.sync.dma_start(out=w2t_sb, in_=w2t.ap())
            b2_sb = singles.tile([C, 4], F32)
            nc.sync.dma_start(out=b2_sb, in_=b2c.ap())
            pt_sb = singles.tile([C, 4, K], F32)
            nc.sync.dma_start(out=pt_sb, in_=pt.ap())
            kb_sb = singles.tile([K, C], F32)
            nc.sync.dma_start(out=kb_sb, in_=kb.ap())
            # kernel bank split per-k so aggregation isn't gated on one 4.7MB DMA
            kw_sb = singles.tile([C, K, KS * KS, C], F32)
            for k in range(K):
                nc.sync.dma_start(out=kw_sb[:, k], in_=kw.ap()[:, k])

            # ---- pooled sums (mean folded into relu scale later) ----
            # split across DVE and ACT so the 4 reduces serialize half as long
            pooled = singles.tile([C, BL], F32)
            junk = singles.tile([C, NPIX], F32)
            if "reduce" in probe_skip:
                nc.vector.memset(pooled, 1.0)
            else:
                for s in range(BL):
                    if s % 2 == 0:
                        nc.vector.tensor_reduce(
                            pooled[:, s : s + 1],
                            x_sb[s].bitcast(F32),
                            axis=mybir.AxisListType.XY,
                            op=mybir.AluOpType.add,
                        )
                    else:
                        nc.scalar.activation(
                            junk,
                            x_sb[s].bitcast(F32).rearrange("p a b -> p (a b)"),
                            mybir.ActivationFunctionType.Copy,
                            accum_out=pooled[:, s : s + 1],
                        )

            # ---- attention MLP in two 2-sample pipelines: samples 0-1 reach
            # alphas (and start convs) without waiting for samples 2-3 ----
            skip_mlp = "mlp" in probe_skip
            h_sb = singles.tile([C, 4, BL], F32)
            s_sb = singles.tile([C, 4, BL], F32)
            alpha_bc = singles.tile([C, BL, K], F32)
            alpha_k8 = singles.tile([K, BL], F32)
            aggb_sb = singles.tile([C, BL], F32)
            if skip_mlp:
                nc.vector.memset(alpha_bc, 0.125)
                nc.vector.memset(alpha_k8, 0.125)
                nc.vector.memset(aggb_sb, 0.0)
            for pr in [] if skip_mlp else range(2):
                sl = slice(2 * pr, 2 * pr + 2)
                ps_h = mlpp.tile([C, 4, 2], F32, tag="ps_mlp")
                for c in range(4):
                    nc.tensor.matmul(
                        ps_h[:, c, :], w1t_sb[:, 128 * c : 128 * (c + 1)],
                        pooled[:, sl], start=True, stop=True,
                    )
                    nc.scalar.activation(
                        h_sb[:, c, sl], ps_h[:, c, :],
                        mybir.ActivationFunctionType.Relu,
                        bias=b1_sb[:, c : c + 1], scale=1.0 / (H * W),
                    )
                ps_s = mlpp.tile([C, 4, 2], F32, tag="ps_mlp")
                for c2 in range(4):
                    for c in range(4):
                        nc.tensor.matmul(
                            ps_s[:, c2, :],
                            w2t_sb[:, c, 128 * c2 : 128 * (c2 + 1)],
                            h_sb[:, c, sl],
                            start=(c == 0), stop=(c == 3),
                        )
                    nc.scalar.activation(
                        s_sb[:, c2, sl], ps_s[:, c2, :],
                        mybir.ActivationFunctionType.Identity,
                        bias=b2_sb[:, c2 : c2 + 1],
                    )
                ps_sc = mlpp.tile([2, K], F32, tag="ps_sm")
                for c2 in range(4):
                    nc.tensor.matmul(
                        ps_sc, s_sb[:, c2, sl], pt_sb[:, c2, :],
                        start=(c2 == 0), stop=(c2 == 3),
                    )
                negmx = scr.tile([2, 1], F32, tag="negmx")
                nc.vector.tensor_reduce(
                    negmx, ps_sc, axis=mybir.AxisListType.X,
                    op=mybir.AluOpType.max, negate=True,
                )
                ex = scr.tile([2, K], F32, tag="ex")
                nc.scalar.activation(
                    ex, ps_sc, mybir.ActivationFunctionType.Exp, bias=negmx,
                )
                sm = scr.tile([2, 1], F32, tag="sm")
                nc.vector.tensor_reduce(
                    sm, ex, axis=mybir.AxisListType.X, op=mybir.AluOpType.add
                )
                rsm = scr.tile([2, 1], F32, tag="rsm")
                nc.vector.reciprocal(rsm, sm)
                alphas = scr.tile([2, K], F32, tag="alphas")
                nc.vector.tensor_scalar_mul(alphas, ex, rsm)

                nc.sync.dma_start(out=alpha_dram.ap()[sl], in_=alphas)
                nc.sync.dma_start(
                    out=alpha_bc[:, sl, :],
                    in_=alpha_dram.ap()[sl].rearrange("b k -> (b k)").unsqueeze(0)
                    .to_broadcast((C, 2 * K))
                    .rearrange("p (b k) -> p b k", b=2),
                )
                nc.sync.dma_start(
                    out=alpha_k8[:, sl],
                    in_=alpha_dram.ap()[sl].rearrange("b k -> k b"),
                )
                ps_ab = mlpp.tile([C, 2], F32, tag="ps_sm")
                nc.tensor.matmul(ps_ab, kb_sb, alpha_k8[:, sl], start=True, stop=True)
                nc.scalar.copy(aggb_sb[:, sl], ps_ab)

            # ---- per sample: aggregate kernel bank, conv, bias, store ----
            taps = [(ti, tj) for ti in range(KS) for tj in range(KS)]
            for s in range(BL):
                if "agg" in probe_skip:
                    aggw = aggpool.tile([C, KS * KS, C], F32R, tag="aggw")
                    nc.vector.tensor_copy(aggw, kw_sb[:, 0])
                    o_sb = opool.tile([C, H, W], F32, tag="out")
                    for chunk in range(NCHUNK):
                        h0 = chunk * RCHUNK
                        ps_c = convp.tile([C, RCHUNK, W], F32, tag="ps_c")
                        for t, (ti, tj) in enumerate(taps):
                            nc.tensor.matmul(
                                ps_c, aggw[:, t, :],
                                x_sb[s][:, h0 + ti : h0 + ti + RCHUNK, tj : tj + W],
                                start=(t == 0), stop=(t == KS * KS - 1),
                            )
                        if "evict_dve" in probe_skip:
                            nc.vector.tensor_scalar_add(
                                o_sb[:, h0 : h0 + RCHUNK, :], ps_c,
                                aggb_sb[:, s : s + 1],
                            )
                        else:
                            nc.scalar.activation(
                                o_sb[:, h0 : h0 + RCHUNK, :], ps_c,
                                mybir.ActivationFunctionType.Identity,
                                bias=aggb_sb[:, s : s + 1],
                            )
                    nc.sync.dma_start(
                        out=out.ap()[s], in_=o_sb.rearrange("p a b -> p (a b)")
                    )
                    continue
                # weighted sum of 8 kernel banks on DVE
                sA = aggpool.tile([C, KS * KS, C], F32, tag="aggA")
                sB = aggpool.tile([C, KS * KS, C], F32, tag="aggB")
                pp = [sA, sB]
                nc.vector.tensor_scalar_mul(
                    sA, kw_sb[:, 0], alpha_bc[:, s, 0:1]
                )
                for k in range(1, K - 1):
                    nc.vector.scalar_tensor_tensor(
                        pp[k % 2], kw_sb[:, k], alpha_bc[:, s, k : k + 1],
                        pp[(k + 1) % 2],
                        op0=mybir.AluOpType.mult, op1=mybir.AluOpType.add,
                    )
                aggw = aggpool.tile([C, KS * KS, C], F32R, tag="aggw")
                nc.vector.scalar_tensor_tensor(
                    aggw, kw_sb[:, K - 1], alpha_bc[:, s, K - 1 : K],
                    pp[(K - 2) % 2],
                    op0=mybir.AluOpType.mult, op1=mybir.AluOpType.add,
                )

                o_sb = opool.tile([C, H, W], F32, tag="out")
                for chunk in range(NCHUNK):
                    h0 = chunk * RCHUNK
                    ps_c = convp.tile([C, RCHUNK, W], F32, tag="ps_c")
                    for t, (ti, tj) in enumerate(taps):
                        nc.tensor.matmul(
                            ps_c,
                            aggw[:, t, :],
                            x_sb[s][:, h0 + ti : h0 + ti + RCHUNK, tj : tj + W],
                            start=(t == 0), stop=(t == KS * KS - 1),
                        )
                    nc.scalar.activation(
                        o_sb[:, h0 : h0 + RCHUNK, :], ps_c,
                        mybir.ActivationFunctionType.Identity,
                        bias=aggb_sb[:, s : s + 1],
                    )
                nc.sync.dma_start(
                    out=out.ap()[s], in_=o_sb.rearrange("p a b -> p (a b)")
                )

    nc.compile()
    return nc


_NC = None


def _get_nc():
    global _NC
    if _NC is None:
        _NC = build()
    return _NC


def prep_inputs(x, prompt_param, w1, b1, w2, b2, kernels_weights, kernels_bias):
    """Host-side layout transforms -> per-core in_maps."""
    x = np.asarray(x, np.float32)
    prompt = np.asarray(prompt_param, np.float32)[0]          # (K, HID)
    w1 = np.asarray(w1, np.float32)
    b1 = np.asarray(b1, np.float32)
    w2 = np.asarray(w2, np.float32)
    b2 = np.asarray(b2, np.float32)
    kwt = np.asarray(kernels_weights, np.float32)             # (K, C, C, 3, 3)
    kbt = np.asarray(kernels_bias, np.float32)                # (K, C)

    w1t = np.ascontiguousarray(w1.T)                          # (C, HID)
    b1c = np.ascontiguousarray(b1.reshape(4, C).T)            # (C, 4)
    w2t = np.ascontiguousarray(w2.T.reshape(4, C, HID).transpose(1, 0, 2))
    b2c = np.ascontiguousarray(b2.reshape(4, C).T)
    pt = np.ascontiguousarray(prompt.T.reshape(4, C, K).transpose(1, 0, 2))
    kw = np.ascontiguousarray(kwt.transpose(2, 0, 3, 4, 1).reshape(C, K, KS * KS, C))
    kb = np.ascontiguousarray(kbt)

    in_maps = []
    for c in range(NCORES):
        xs = x[c * BL : (c + 1) * BL]                          # (4, C, H, W)
        xpad = np.zeros((BL, C, HP, WP), np.float32)
        xpad[:, :, 1 : H + 1, 1 : W + 1] = xs
        xpad = round_tf32(xpad).reshape(BL, C, NPIX)
        in_maps.append(
            {
                "xp": xpad, "w1t": w1t, "b1c": b1c, "w2t": w2t, "b2c": b2c,
                "pt": pt, "kb": kb, "kw": kw,
            }
        )
    return in_maps


def kernel(**inputs) -> np.ndarray:
    nc = _get_nc()
    in_maps = prep_inputs(**inputs)
    res = run_bass_kernel_spmd(nc, in_maps, core_ids=list(range(NCORES)))
    outs = [res.results[c]["out"].reshape(BL, C, H, W) for c in range(NCORES)]
    return np.concatenate(outs, axis=0)


if __name__ == "__main__":
    import reference

    inputs = {k: np.asarray(v) for k, v in reference.setup_inputs().items()}
    expected = np.asarray(reference.reference(**inputs))
    actual = kernel(**inputs)
    scale = np.abs(expected).max()
    err = np.abs(actual - expected).max()
    print(f"absmax={err:.3e} scale={scale:.3f} rel={err / scale:.3e}")
